# revision 29
# baseline (speedup 1.0000x reference)
"""Trainium2 Bass kernel for nn_AttentionalGNN_81982335746601.

Computation (reference semantics, full shapes):
  desc0 (1,128,128), desc1 (1,128,2048), dist (1,128,128,2048)
  layer0: desc{0,1} += AttentionalPropagation_self(desc{0,1})  [shared weights]
  layer1: out[p, i, j] = (cw2 @ relu(cw1 @ [q_i; k_j; d_ij] + cb1) + cb2)
          -> (128, 2048, 128), softmax-free.

Sharding: core p takes query rows i in [256p, 256p+256).  dist slice
dist[0, 16p:16p+16, :, :] covers exactly that range; no cross-core comms.

v2 design (cost-model-driven):
 * dist and out move as bf16 (halves the exclusive-DMA-device busy time;
   numpy-verified max rel err 3.6e-3 vs the 2e-2 gate).
 * phase B runs KEY-major: one 256-col group per key j.  The per-key term
   Bc_j = cwk@k_j + cb1 is a per-partition scalar column, the per-query
   term A_i = cwq@q_i is one fixed 256-wide tensor.  A single DVE/Pool
   scalar_tensor_tensor computes m = max(hp + Bc_j, -A) which equals
   relu(z) - A; the linear -A correction is folded into the output as
   OA = cw2@A and added on the HOST (along with cb2) after gathering.
 * per-pair work on device: 2 bf16 matmuls (cwd, cw2) + 1 stt (DVE or
   Pool, alternating) + 1 psum->bf16 copy per 4 groups (mostly ACT).
 * phase A (f32r): softmax denominator via a ones-column appended to V
   (row 32 of each 33-row PV psum = sum of exp scores) instead of a
   128-wide ones matmul; q/k biases folded into DVE/Pool copies;
   v-bias folded into wm's bias host-side (softmax rows sum to 1).
 * DMAs are few and large (4 dist-in, 4 out, ~7 const) so the HWDGE /
   sequencer issue cost stays off the critical path.
"""

import numpy as np
import ml_dtypes
from contextlib import ExitStack

import concourse.bacc as bacc
import concourse.mybir as mybir
from concourse.tile import TileContext
from concourse.bass_utils import run_bass_kernel_spmd

F32 = mybir.dt.float32
F32R = mybir.dt.float32r
BF16 = mybir.dt.bfloat16
AF = mybir.ActivationFunctionType
ALU = mybir.AluOpType

D = 128
H = 4
DH = 32
N0 = 128
N1 = 2048
NCORES = 8
NQL = N1 // NCORES            # 256 local query nodes
SCALE = float(1.0 / np.sqrt(DH))

# packed f32r weight blocks
_WNAMES_W = ["wkT", "wqT0", "wqT1", "wqT2", "wqT3"]   # then WVX, d0, d1loc
_WNAMES_C = ["wmT", "w1T00", "w1T10", "w1T01", "w1T11",
             "w2T0", "w2T1", "cwq", "cwk", "cw2f", "idn"]
VXC = 33 * H                         # 132 cols per V chunk
BIGW_COLS = 5 * D + VXC + D + NQL    # 1156
BIGC_COLS = 11 * D                   # 1408

# phase-B per-4-key-unit modes: 'T' = B-term via 1-row PE matmul + one
# 1024-wide DVE tensor_tensor max; 'S' = per-key DVE scalar_tensor_tensor;
# 'H' = A-term via PE identity matmul + per-key ACT Relu (bias=Bc).
_S_UNITS = {0, 3, 6, 9, 12, 15, 18, 21, 24, 28, 30}
_H_UNITS = {5, 16, 27}
MODES = ['H' if u in _H_UNITS else ('S' if u in _S_UNITS else 'T')
         for u in range(32)]
H_KEYS = [4 * u + t for u in _H_UNITS for t in range(4)]

_CACHE: dict = {}


def _build(trace_sim: bool = False, debug_taps: bool = False):
    nc = bacc.Bacc("TRN2", target_bir_lowering=False, debug=False,
                   num_devices=NCORES)

    bigw = nc.dram_tensor("bigw", [D, BIGW_COLS], F32,
                          kind="ExternalInput").ap()
    bigc = nc.dram_tensor("bigc", [D, BIGC_COLS], F32,
                          kind="ExternalInput").ap()
    d1d = nc.dram_tensor("d1d", [D, N1], F32, kind="ExternalInput").ap()
    bigf = nc.dram_tensor("bigf", [D, 16], F32, kind="ExternalInput").ap()
    rows = nc.dram_tensor("rows", [1, 1024], F32, kind="ExternalInput").ap()
    wb16 = nc.dram_tensor("wb16", [D, 2 * D], BF16, kind="ExternalInput").ap()
    dist2 = nc.dram_tensor("dist2", [D, N0 * NQL], BF16,
                           kind="ExternalInput").ap()
    outd = nc.dram_tensor("out", [D, N0 * NQL], BF16,
                          kind="ExternalOutput").ap()
    oad = nc.dram_tensor("oa", [D, NQL], F32, kind="ExternalOutput").ap()

    with TileContext(nc, trace_sim=trace_sim) as tc:
        with ExitStack() as st:
            cp = st.enter_context(tc.tile_pool(name="consts", bufs=1))
            ap_ = st.enter_context(tc.tile_pool(name="apool", bufs=1))

            BIGW = cp.tile([D, BIGW_COLS], F32R, name="BIGW")
            nc.sync.dma_start(out=BIGW[:], in_=bigw[:].bitcast(F32R))
            D1 = cp.tile([D, N1], F32R, name="D1")
            nc.sync.dma_start(out=D1[:], in_=d1d[:].bitcast(F32R))
            BIGF = cp.tile([D, 16], F32, name="BIGF")
            nc.sync.dma_start(out=BIGF[:], in_=bigf[:])
            ROWS = cp.tile([1, 1024], F32R, name="ROWS")
            nc.sync.dma_start(out=ROWS[:], in_=rows[:].bitcast(F32R))
            BIGC = cp.tile([D, BIGC_COLS], F32R, name="BIGC")
            nc.sync.dma_start(out=BIGC[:], in_=bigc[:].bitcast(F32R))
            WB = cp.tile([D, 2 * D], BF16, name="WB")
            nc.sync.dma_start(out=WB[:], in_=wb16[:])
            # dist streamed in 8 chunks of 16 keys; pool opened early so
            # the first chunks' DMAs hoist to t=0 and overlap phase A
            dbp = st.enter_context(tc.tile_pool(name="dbp", bufs=3))

            W = {}
            for i, nm_ in enumerate(_WNAMES_W):
                W[nm_] = BIGW[:, D * i:D * (i + 1)]
            for i, nm_ in enumerate(_WNAMES_C):
                W[nm_] = BIGC[:, D * i:D * (i + 1)]
            WVX = BIGW[:, 5 * D:5 * D + VXC]
            D0 = BIGW[:, 5 * D + VXC:6 * D + VXC]
            D1L = BIGW[:, 6 * D + VXC:6 * D + VXC + NQL]
            WD16 = WB[:, 0:D]
            W216 = WB[:, D:2 * D]
            # f32 bias columns in BIGF
            BM = BIGF[:, 0:1]
            B1T = BIGF[:, 1:2]
            B1B = BIGF[:, 2:3]
            B2 = BIGF[:, 3:4]
            CB1 = BIGF[:, 4:5]
            BKC = BIGF[:, 5:6]
            BQC = [BIGF[:, 6 + h:7 + h] for h in range(H)]
            ONE128 = ROWS[:, 0:128]
            ONE32 = ROWS[:, 0:32]
            ONE256 = ROWS[:, 0:256]
            OSHOT = ROWS[:, 256:256 + VXC]
            CB1R = ROWS[:, 388:516]

            stP = ExitStack()
            psA = stP.enter_context(tc.tile_pool(name="psA", bufs=1,
                                                 space="PSUM"))
            sm = stP.enter_context(tc.tile_pool(name="smlp", bufs=2))
            ptp = stP.enter_context(tc.tile_pool(name="ptp", bufs=2))

            def conv_stage(x_full, x_q, n_kv, n_q, tagn):
                """q/k/v convolutions.  K/Q biases folded into the Pool-engine
                psum->sbuf copies; V gets a ones column per head (col 33h+32)
                for the softmax denominator; v-bias is folded into bm
                host-side."""
                nm = n_kv // 128
                K = ap_.tile([D, n_kv], F32R, name=f"K{tagn}")
                QH = []
                VTX = ap_.tile([D, VXC * nm], F32R, name=f"VTX{tagn}")
                for c0 in range(0, n_kv, 512):
                    w = min(512, n_kv - c0)
                    pk = psA.tile([D, 512], F32, name="pk", tag="sg",
                                  bufs=2)[:, :w]
                    nc.tensor.matmul(pk, W["wkT"], x_full[:, c0:c0 + w],
                                     start=True, stop=True)
                    nc.vector.tensor_scalar_add(K[:, c0:c0 + w], pk, BKC)
                for h in range(H):
                    pq = psA.tile([D, 512], F32, name="pq", tag="sg",
                                  bufs=2)[:, :n_q]
                    nc.tensor.matmul(pq, W[f"wqT{h}"], x_q, start=True,
                                     stop=True)
                    Qh = ap_.tile([D, 256], F32R,
                                  name=f"Q{tagn}{h}")[:, :n_q]
                    nc.vector.tensor_scalar_add(Qh, pq, BQC[h])
                    QH.append(Qh)
                for j in range(nm):
                    pv = psA.tile([D, 512], F32, name="pv", tag="sg",
                                  bufs=2)[:, :VXC]
                    nc.tensor.matmul(pv, x_full[:, 128 * j:128 * j + 128],
                                     WVX, start=True, stop=False)
                    nc.tensor.matmul(pv, ONE128, OSHOT, start=False,
                                     stop=True)
                    nc.vector.tensor_copy(VTX[:, VXC * j:VXC * (j + 1)], pv)
                return K, QH, VTX

            def prop(stage, x_q, n_kv, n_q, tagn):
                """Attention + MLP; returns x_q + MLP update (f32r)."""
                nm = n_kv // 128
                K, QH, VTX = stage
                OM = ap_.tile([D, n_q], F32R, name=f"OM{tagn}")
                POH = [psA.tile([33, 512], F32, name=f"po{h}")[:, :n_q]
                       for h in range(H)]
                nsg = (H * n_q + 511) // 512        # 512-wide score groups
                hpg = 512 // n_q                    # heads per group
                for j in range(nm):
                    PTs = []
                    for gi in range(nsg):
                        psg = psA.tile([D, 512], F32, name=f"psg{gi}",
                                       tag="sg", bufs=2)
                        for hh in range(hpg):
                            h = gi * hpg + hh
                            nc.tensor.matmul(
                                psg[:, hh * n_q:(hh + 1) * n_q],
                                K[:, 128 * j:128 * j + 128],
                                QH[h], start=True, stop=True)
                        PT = ptp.tile([D, 512], F32R, name="pt")
                        nc.scalar.activation(PT[:], psg[:], AF.Exp)
                        PTs.append(PT)
                    for h in range(H):
                        PT = PTs[h // hpg]
                        nc.tensor.matmul(
                            POH[h],
                            VTX[:, VXC * j + 33 * h:VXC * j + 33 * h + 33],
                            PT[:, (h % hpg) * n_q:(h % hpg + 1) * n_q],
                            start=(j == 0), stop=(j == nm - 1))
                # denominators live in row 32 of each POH; gather, replicate
                # to 32 partitions via a 1-row matmul, reciprocal, merge.
                RROW = ap_.tile([1, H * n_q], F32R, name=f"rr{tagn}")
                for h in range(H):
                    nc.vector.tensor_copy(RROW[:, h * n_q:(h + 1) * n_q],
                                          POH[h][32:33, :])
                RBLK = psA.tile([32, 1024], F32, name="rblk")[:, :H * n_q]
                for c0 in range(0, H * n_q, 512):
                    w = min(512, H * n_q - c0)
                    nc.tensor.matmul(RBLK[:, c0:c0 + w], ONE32,
                                     RROW[:, c0:c0 + w], start=True,
                                     stop=True)
                RI = ap_.tile([32, H * 256], F32, name=f"RI{tagn}")[:, :H * n_q]
                nc.vector.reciprocal(RI, RBLK)
                for h in range(H):
                    nc.vector.tensor_mul(OM[DH * h:DH * h + DH, :],
                                         POH[h][0:32, :],
                                         RI[:, h * n_q:(h + 1) * n_q])
                # msg + MLP epilogue
                DN = ap_.tile([D, n_q], F32R, name=f"DN{tagn}")
                pm = psA.tile([D, 256], F32, name="pm", tag="sg",
                              bufs=2)[:, :n_q]
                nc.tensor.matmul(pm, W["wmT"], OM[:], start=True, stop=True)
                MSG = sm.tile([D, 256], F32R, name="msg")[:, :n_q]
                nc.vector.tensor_scalar_add(MSG, pm, BM)
                ph1 = psA.tile([D, 256], F32, name="pm", tag="sg",
                               bufs=2)[:, :n_q]
                nc.tensor.matmul(ph1, W["w1T00"], x_q, start=True, stop=False)
                nc.tensor.matmul(ph1, W["w1T10"], MSG, start=False, stop=True)
                HT = sm.tile([D, 256], F32R, name="ht")[:, :n_q]
                nc.vector.tensor_scalar(HT, ph1, B1T, 0.0, op0=ALU.add,
                                        op1=ALU.max)
                ph2 = psA.tile([D, 256], F32, name="pm", tag="sg",
                               bufs=2)[:, :n_q]
                nc.tensor.matmul(ph2, W["w1T01"], x_q, start=True, stop=False)
                nc.tensor.matmul(ph2, W["w1T11"], MSG, start=False, stop=True)
                HB = sm.tile([D, 256], F32R, name="hb")[:, :n_q]
                nc.vector.tensor_scalar(HB, ph2, B1B, 0.0, op0=ALU.add,
                                        op1=ALU.max)
                py = psA.tile([D, 256], F32, name="pm", tag="sg",
                              bufs=2)[:, :n_q]
                nc.tensor.matmul(py, W["w2T0"], HT, start=True, stop=False)
                nc.tensor.matmul(py, W["w2T1"], HB, start=False, stop=True)
                nc.vector.scalar_tensor_tensor(DN[:], py, B2, x_q,
                                               op0=ALU.add, op1=ALU.add)
                return DN

            st0 = conv_stage(D0, D0, N0, N0, "0")
            st1 = conv_stage(D1, D1L, N1, NQL, "1")
            DN0 = prop(st0, D0, N0, N0, "0")
            DN1 = prop(st1, D1L, N1, NQL, "1")

            # ---- phase B prep: per-key column Bc (and its transpose),
            # per-query tensors A (negated+tiled), OA = cw2@A ----
            pb = psA.tile([D, 512], F32, name="pk", tag="sg", bufs=2)[:, :N0]
            nc.tensor.matmul(pb, W["cwk"], DN0[:], start=True, stop=True)
            BC = ap_.tile([D, N0], F32, name="BC")
            nc.scalar.activation(BC[:], pb, AF.Identity, bias=CB1)
            pbt = psA.tile([D, 512], F32, name="pk", tag="sg", bufs=2)[:, :D]
            nc.tensor.matmul(pbt, DN0[:], W["cwk"], start=True, stop=False)
            nc.tensor.matmul(pbt, ONE128, CB1R, start=False, stop=True)
            BCT = ap_.tile([D, D], BF16, name="BCT")
            nc.vector.tensor_copy(BCT[:], pbt)
            # flatten keys-on-partitions -> one bf16 row so 1-row matmul
            # lhsT slices start at partition 0
            BCR = ap_.tile([1, D * D], BF16, name="BCR")
            nc.sync.dma_start(
                out=BCR[:].rearrange("o (j p) -> o j p", j=D),
                in_=BCT[:])
            ONEB = ap_.tile([1, NQL], BF16, name="ONEB")
            nc.vector.memset(ONEB[:], 1.0)
            pa = psA.tile([D, 512], F32, name="pk", tag="sg", bufs=2)[:, :NQL]
            nc.tensor.matmul(pa, W["cwq"], DN1[:], start=True, stop=True)
            NAQ4 = ap_.tile([D, 4 * NQL], BF16, name="NAQ4")
            nc.scalar.activation(NAQ4[:, 0:NQL], pa, AF.Copy, scale=-1.0)
            nc.vector.tensor_copy(NAQ4[:, NQL:2 * NQL], NAQ4[:, 0:NQL])
            nc.vector.tensor_copy(NAQ4[:, 2 * NQL:4 * NQL], NAQ4[:, 0:2 * NQL])
            AQF = ap_.tile([D, NQL], F32R, name="AQF")
            nc.vector.tensor_copy(AQF[:], pa)
            po = psA.tile([D, 512], F32, name="pk", tag="sg", bufs=2)[:, :NQL]
            nc.tensor.matmul(po, W["cw2f"], AQF[:], start=True, stop=True)
            OAS = ap_.tile([D, NQL], F32, name="OAS")
            nc.vector.tensor_copy(OAS[:], po)
            nc.sync.dma_start(out=oad[:], in_=OAS[:])

            if debug_taps:
                for nm_, t_ in [("DN0", DN0), ("DN1", DN1), ("BC", BC),
                                ("NAQ", NAQ), ("OAS", OAS)]:
                    dbg = nc.dram_tensor(f"dbg_{nm_}", list(t_.shape),
                                         t_.tensor.dtype,
                                         kind="ExternalOutput").ap()
                    nc.sync.dma_start(out=dbg[:], in_=t_[:])

            stP.close()

            # ---- phase B: key-major cross MLP over pair columns ----
            # group j (key) covers out cols [256j, 256j+256) = all local
            # queries.  Per 4-key unit: 4 cwd matmuls into one psum tile,
            # ONE wide psum->bf16 drain (ACT/DVE split), per-key stt in
            # SBUF (mostly Pool, which cannot touch psum), 4 cw2 matmuls,
            # one wide output drain.  m = max(hs + Bc_j, -A); host adds
            # OA + cb2.
            with (
                tc.tile_pool(name="stgp", bufs=2) as stgp,
                tc.tile_pool(name="mp", bufs=8) as mp,
                tc.tile_pool(name="psB", bufs=1, space="PSUM") as psB,
            ):
                STG = None
                DBC = None
                for u in range(N0 // 4):
                    mode = MODES[u]
                    if u % 4 == 0:
                        STG = stgp.tile([D, 4096], BF16, name="stg")
                    if u % 8 == 0:
                        c = u // 8
                        DBC = dbp.tile([D, 8192], BF16, name="dbc")
                        nc.sync.dma_start(
                            out=DBC[:],
                            in_=dist2[:, 8192 * c:8192 * (c + 1)])
                    hp = psB.tile([D, 1024], F32, name="hp", tag="hp",
                                  bufs=2)
                    for t in range(4):
                        j = 4 * u + t
                        jc = 256 * (j % 32)
                        sl = slice(256 * t, 256 * t + 256)
                        nc.tensor.matmul(hp[:, sl], WD16,
                                         DBC[:, jc:jc + 256],
                                         start=True, stop=(mode == 'S'))
                        if mode == 'T':
                            nc.tensor.matmul(hp[:, sl],
                                             BCR[:, D * j:D * (j + 1)],
                                             ONEB[:], start=False, stop=True)
                        elif mode == 'H':
                            nc.tensor.matmul(hp[:, sl], W["idn"], AQF[:],
                                             start=False, stop=True)
                    ms = []
                    if mode == 'T':
                        M4 = mp.tile([D, 1024], BF16, name="m4", tag="m4",
                                     bufs=2)
                        nc.vector.tensor_tensor(M4[:], hp[:], NAQ4[:],
                                                op=ALU.max)
                        ms = [M4[:, 256 * t:256 * t + 256] for t in range(4)]
                    else:
                        for t in range(4):
                            j = 4 * u + t
                            sl = slice(256 * t, 256 * t + 256)
                            m = mp.tile([D, NQL], BF16, name="m", tag="m",
                                        bufs=6)
                            if mode == 'S':
                                nc.vector.scalar_tensor_tensor(
                                    m[:], hp[:, sl], BC[:, j:j + 1],
                                    NAQ4[:, 0:NQL],
                                    op0=ALU.add, op1=ALU.max)
                            else:
                                nc.scalar.activation(m[:], hp[:, sl],
                                                     AF.Relu,
                                                     bias=BC[:, j:j + 1])
                            ms.append(m[:])
                    op = psB.tile([D, 1024], F32, name="op", tag="op",
                                  bufs=2)
                    for t in range(4):
                        nc.tensor.matmul(op[:, 256 * t:256 * t + 256], W216,
                                         ms[t], start=True, stop=True)
                    ssl = slice(1024 * (u % 4), 1024 * (u % 4) + 1024)
                    nc.scalar.activation(STG[:, ssl], op[:], AF.Copy)
                    if u % 4 == 3:
                        r = u // 4
                        nc.sync.dma_start(
                            out=outd[:, 4096 * r:4096 * (r + 1)],
                            in_=STG[:])

    nc.compile()
    return nc


def _host_prep(inputs):
    g = {k: np.asarray(v, dtype=np.float32) for k, v in inputs.items()}
    perm = np.empty(D, dtype=np.int64)
    for h in range(H):
        for d in range(DH):
            perm[DH * h + d] = H * d + h

    w1T = g["a_w1"].T
    w2T = g["a_w2"].T
    cw1T = g["c_w1"].T
    wqTp = g["a_wq"].T[:, perm] * SCALE
    wvTp = g["a_wv"].T[:, perm]
    wvx = np.zeros((D, VXC), dtype=np.float32)
    for h in range(H):
        wvx[:, 33 * h:33 * h + 32] = wvTp[:, DH * h:DH * (h + 1)]
    blocks = {
        "wkT": g["a_wk"].T[:, perm],
        "wmT": g["a_wm"].T[perm, :],
        "w1T00": w1T[0:D, 0:D], "w1T10": w1T[D:2 * D, 0:D],
        "w1T01": w1T[0:D, D:2 * D], "w1T11": w1T[D:2 * D, D:2 * D],
        "w2T0": w2T[0:D, :], "w2T1": w2T[D:2 * D, :],
        "cwq": cw1T[0:D, :], "cwk": cw1T[D:2 * D, :],
        "cw2f": g["c_w2"].T,
        "idn": np.eye(D, dtype=np.float32),
    }
    for h in range(H):
        mm = np.zeros((D, D), dtype=np.float32)
        mm[:, DH * h:DH * (h + 1)] = wqTp[:, DH * h:DH * (h + 1)]
        blocks[f"wqT{h}"] = mm

    bigc = np.ascontiguousarray(
        np.concatenate([blocks[nm_] for nm_ in _WNAMES_C], axis=1))

    bigf = np.zeros((D, 16), dtype=np.float32)
    bigf[:, 0] = g["a_bm"] + g["a_wm"] @ g["a_bv"]
    bigf[:, 1] = g["a_b1"][0:D]
    bigf[:, 2] = g["a_b1"][D:2 * D]
    bigf[:, 3] = g["a_b2"]
    bigf[:, 4] = g["c_b1"]
    bigf[:, 5] = g["a_bk"][perm]
    bqp = g["a_bq"][perm] * SCALE
    for h in range(H):
        bigf[DH * h:DH * (h + 1), 6 + h] = bqp[DH * h:DH * (h + 1)]

    rows_ = np.zeros((1, 1024), dtype=np.float32)
    rows_[0, 0:256] = 1.0
    for h in range(H):
        rows_[0, 256 + 33 * h + 32] = 1.0
    rows_[0, 388:516] = g["c_b1"]

    wb = np.concatenate([cw1T[2 * D:3 * D, :], g["c_w2"].T],
                        axis=1).astype(ml_dtypes.bfloat16)

    d0 = g["desc0"][0]
    d1 = g["desc1"][0]
    dist = g["dist"][0]
    d1c = np.ascontiguousarray(d1)
    in_maps = []
    for p in range(NCORES):
        bigw = np.concatenate(
            [blocks[nm_] for nm_ in _WNAMES_W]
            + [wvx, d0, d1[:, NQL * p:NQL * (p + 1)]], axis=1)
        ds = dist[16 * p:16 * (p + 1)]          # (16, 128ch, 2048)
        dist2 = np.ascontiguousarray(
            ds.reshape(16, D, 16, 128).transpose(1, 3, 0, 2)
            .reshape(D, N0 * NQL)).astype(ml_dtypes.bfloat16)
        in_maps.append({
            "bigw": np.ascontiguousarray(bigw),
            "bigc": bigc,
            "d1d": d1c,
            "bigf": bigf,
            "rows": rows_,
            "wb16": np.ascontiguousarray(wb),
            "dist2": dist2,
        })
    return in_maps


def kernel(**inputs):
    if "nc" not in _CACHE:
        _CACHE["nc"] = _build()
    nc = _CACHE["nc"]
    in_maps = _host_prep(inputs)
    res = run_bass_kernel_spmd(nc, in_maps, list(range(NCORES))).results
    cb2 = np.asarray(inputs["c_b2"], dtype=np.float32)
    # hybrid ('H') keys already include the A-term on device; others get
    # OA = cw2@A added here (the -A correction of the max trick).
    oa_mask = np.ones(N0, dtype=np.float32)
    oa_mask[H_KEYS] = 0.0
    parts = []
    for p in range(NCORES):
        stage = np.asarray(res[p]["out"]).astype(np.float32)
        oa = np.asarray(res[p]["oa"]).astype(np.float32)
        # stage[p, j*256 + i] -> [p, i, j]; add host-side linear terms
        cur = stage.reshape(D, N0, NQL).transpose(0, 2, 1) \
            + oa[:, :, None] * oa_mask[None, None, :] + cb2[:, None, None]
        parts.append(cur)
    full = np.concatenate(parts, axis=1)
    return full.astype(np.float32)
